# revision 28
# baseline (speedup 1.0000x reference)
"""Causal self-attention (B=2, T=2048, C=1024, H=16) on 8 trn2 NeuronCores.

Sharding (Megatron-style, per spec hint):
  - tensor-parallel over heads: core p owns heads {2p, 2p+1}.  Each core
    computes Q^T/K^T/V^T for its 2 heads from the full x, then causal
    attention (streaming softmax without max-subtraction; the denominator
    comes from a ones-column appended to V).
  - per batch: an AllToAll redistributes that batch's attention outputs so
    that core p holds all 1024 channels for the batch's tokens
    [256p, 256p+256); batch 0's AllToAll and projection overlap batch 1's
    qkv/attention work.
  - projection: each core computes the full output projection for its two
    256-token slices and writes a disjoint [512, 1024] output block
    (rows b*256+i = batch b, token 256*p+i).

The emission order pipelines per batch so the (in-order) PE never waits on
the 16.8 MB x^T stream: qkv(b0) -> attention(b0) -> qkv(b1) [x tail
streams during attention(b0)] -> attention(b1) -> projections.

Layouts: x/W pre-transposed on host so contractions land on partitions;
scores computed transposed (S^T = K Q^T) so the softmax sum is a matmul
reduction and exp(S^T) feeds O^T = V^T_aug P directly, accumulated in PSUM
over k-tiles; all matmuls in float32r.
"""

import numpy as np

B, T, C, H, D = 2, 2048, 1024, 16, 64
NCORES = 8
HL = H // NCORES        # heads per core = 2
TOK = B * T             # 4096 global tokens
TSL = TOK // NCORES     # 512 output tokens per core (256 per batch)
SL = 256                # per-batch token slice per core
P = 128
CT = C // P             # 8 contraction tiles
NQC = T // 512          # 4 q-chunks per batch
NKT = T // P            # 16 k-tiles per batch
KG = 2                  # k-tiles per exp group
SCALE = D ** -0.5

_CACHE = {}


def _build_nc():
    import concourse.bass as bass
    import concourse.mybir as mybir
    from concourse import bacc
    from concourse.tile import TileContext

    f32 = mybir.dt.float32
    f32r = mybir.dt.float32r
    bf16 = mybir.dt.bfloat16
    AF = mybir.ActivationFunctionType
    ALU = mybir.AluOpType

    nc = bacc.Bacc(
        "TRN2", target_bir_lowering=False, debug=False, num_devices=NCORES
    )

    xT = nc.dram_tensor("xT", [C, TOK], bf16, kind="ExternalInput")
    wqkvT = nc.dram_tensor("wqkvT", [C, 3 * P], bf16, kind="ExternalInput")
    bqkv = nc.dram_tensor("bqkv", [3 * P], f32, kind="ExternalInput")
    wpT = nc.dram_tensor("wpT", [C, C], bf16, kind="ExternalInput")
    bp = nc.dram_tensor("bp", [C], bf16, kind="ExternalInput")
    tri = nc.dram_tensor("tri", [P, P], bf16, kind="ExternalInput")
    onesd = nc.dram_tensor("ones", [P, P], bf16, kind="ExternalInput")
    ident = nc.dram_tensor("ident", [P, P], bf16, kind="ExternalInput")
    y = nc.dram_tensor("y", [TSL, C], f32, kind="ExternalOutput")

    with TileContext(nc, num_cores=NCORES) as tc:
        from contextlib import ExitStack

        with ExitStack() as ctx:
            const = ctx.enter_context(tc.tile_pool(name="const", bufs=1))
            persist = ctx.enter_context(tc.tile_pool(name="persist", bufs=1))
            dram = ctx.enter_context(tc.tile_pool(name="dram", bufs=1, space="DRAM"))

            # ---- constants; small ones first so nothing queues behind bulk
            tri_sb = const.tile([P, P], bf16)
            id_sb = const.tile([P, P], bf16)
            bq_sb = const.tile([P, 3], f32)
            bp_sb = const.tile([1, C], bf16)
            ones_sb = const.tile([1, P], bf16)
            ones2_sb = const.tile([P, 2], bf16)
            w_sb = const.tile([P, CT, 3 * P], bf16)     # wqkvT tiles
            wp_sb = const.tile([P, CT, C], bf16)        # W_proj^T (loaded late)
            nc.gpsimd.dma_start(tri_sb[:], tri[:])
            nc.gpsimd.dma_start(id_sb[:], ident[:])
            nc.gpsimd.dma_start(bq_sb[:], bqkv.rearrange("(et p) -> p et", p=P))
            nc.gpsimd.dma_start(bp_sb[:], bp.rearrange("(o c) -> o c", o=1))
            nc.gpsimd.dma_start(ones_sb[:], onesd[0:1, :])
            nc.gpsimd.dma_start(ones2_sb[:], onesd[:, 0:2])
            nc.sync.dma_start(w_sb[:], wqkvT.rearrange("(ct p) e -> p ct e", p=P))

            # ---- persistent activations (per batch for fine-grained deps)
            qTb = [persist.tile([P, T], bf16, name=f"qT{b}") for b in range(B)]
            kTb = [persist.tile([P, T], bf16, name=f"kT{b}") for b in range(B)]
            vTb = [persist.tile([P, T], bf16, name=f"vT{b}") for b in range(B)]
            # V with ones column, per batch: [128 tok, k-tile, 2*65]
            vaugb = [persist.tile([P, NKT, 2 * 65], bf16, name=f"vaug{b}")
                     for b in range(B)]
            # A^T per local head (each head stays at partitions 0-63)
            anorm = [persist.tile([64, TOK], bf16, name=f"anorm{h}")
                     for h in range(HL)]
            ddram = dram.tile([B * HL * NQC, 512], f32)  # raw denominators
            rdram = dram.tile([B * HL, T], f32)          # reciprocals (bounce)

            pools = [
                tc.tile_pool(name="sps", bufs=2, space="PSUM"),
                tc.tile_pool(name="ops", bufs=2, space="PSUM"),
                tc.tile_pool(name="mm", bufs=2, space="PSUM"),
                tc.tile_pool(name="pT", bufs=2),
                tc.tile_pool(name="ds", bufs=2),
                tc.tile_pool(name="rp", bufs=2),
            ]
            sps, ops, mm, ppool, dspool, rppool = (
                ctx.enter_context(p) for p in pools)

            def qkv_chunk(b, tc4):
                """qkv^T for one 512-token chunk of batch b + V transposes."""
                if True:
                    xsl = xpool.tile([P, CT, 512], bf16, tag="x")
                    t0 = b * T + tc4 * 512
                    nc.sync.dma_start(
                        xsl[:],
                        xT[:, t0:t0 + 512].rearrange("(ct p) t -> p ct t", p=P),
                    )
                    for et, dstl in enumerate((qTb, kTb, vTb)):
                        ps = mm.tile([P, 512], f32, tag="mm")
                        for ct in range(CT):
                            nc.tensor.matmul(
                                ps[:],
                                lhsT=w_sb[:, ct, et * P:(et + 1) * P],
                                rhs=xsl[:, ct, :],
                                start=(ct == 0),
                                stop=(ct == CT - 1),
                            )
                        nc.vector.tensor_scalar_add(
                            dstl[b][:, tc4 * 512:(tc4 + 1) * 512],
                            ps[:],
                            bq_sb[:, et:et + 1],
                        )
                    # V^T -> V for this chunk's 4 k-tiles (PE transpose)
                    for kt in range(tc4 * 4, tc4 * 4 + 4):
                        tp = mm.tile([P, P], bf16, tag="mm")
                        nc.tensor.transpose(
                            tp[:],
                            vTb[b][:, kt * P:(kt + 1) * P],
                            id_sb[:],
                        )
                        nc.vector.tensor_copy(
                            vaugb[b][:, kt, 0:2 * 65]
                            .rearrange("p (h e) -> p h e", h=2)[:, :, 0:64],
                            tp.rearrange("p (h e) -> p h e", h=2),
                        )
                        nc.vector.tensor_copy(
                            vaugb[b][:, kt, 64:2 * 65:65], ones2_sb[:]
                        )

            def attention_block(b, h, qc):
                bh = b * HL + h
                hp = slice(64 * h, 64 * h + 64)
                q0 = qc * 512
                nk = 4 * qc + 4                   # causal k-tiles
                ops_t = ops.tile([65, 512], f32, tag="o")
                for g0 in range(0, nk, KG):
                    gn = min(KG, nk - g0)
                    sp = sps.tile([P, KG * 512], f32, tag="s")
                    for j in range(gn):
                        ki = g0 + j
                        nc.tensor.matmul(
                            sp[:, j * 512:(j + 1) * 512],
                            lhsT=kTb[b][hp, ki * P:(ki + 1) * P],
                            rhs=qTb[b][hp, q0:q0 + 512],
                            start=True,
                            stop=True,
                        )
                    pt = ppool.tile([P, KG * 512], bf16, tag="p")
                    nc.scalar.activation(
                        pt[:, 0:gn * 512],
                        sp[:, 0:gn * 512],
                        AF.Exp,
                        scale=SCALE,
                    )
                    for j in range(gn):
                        ki = g0 + j
                        off = ki * P - q0
                        if 0 <= off:
                            nc.vector.tensor_tensor(
                                pt[:, j * 512 + off:j * 512 + off + P],
                                pt[:, j * 512 + off:j * 512 + off + P],
                                tri_sb[:],
                                ALU.mult,
                            )
                        lo = max(0, off)
                        nc.tensor.matmul(
                            ops_t[:, lo:512],
                            lhsT=vaugb[b][:, ki, h * 65:h * 65 + 65],
                            rhs=pt[:, j * 512 + lo:(j + 1) * 512],
                            start=(ki == 0),
                            stop=(ki == nk - 1),
                        )
                # stash unnormalised O^T rows + denominator row
                nc.vector.tensor_copy(
                    anorm[h][:, b * T + q0:b * T + q0 + 512],
                    ops_t[0:64, :],
                )
                dst = dspool.tile([65, 512], f32, tag="ds")
                nc.vector.tensor_copy(dst[64:65, :], ops_t[64:65, :])
                nc.sync.dma_start(
                    ddram[bh * NQC + qc:bh * NQC + qc + 1, :],
                    dst[64:65, :],
                )

            def normalize(b):
                for h in range(HL):
                    bh = b * HL + h
                    dpk = rppool.tile([32, 64], f32, tag="dpk")
                    rpk = rppool.tile([32, 64], f32, tag="rpk")
                    rsc = rppool.tile([32, 64], f32, tag="rsc")
                    nc.sync.dma_start(
                        dpk[:],
                        ddram[bh * NQC:(bh + 1) * NQC, :]
                        .rearrange("u (rr f) -> (u rr) f", f=64),
                    )
                    nc.vector.reciprocal_approx_accurate(rpk[:], dpk[:], rsc[:])
                    nc.sync.dma_start(
                        rdram[bh:bh + 1, :]
                        .rearrange("o (rr f) -> (o rr) f", f=64),
                        rpk[:],
                    )
                    rb = rbpool.tile([64, T], f32, tag="rb")
                    nc.sync.dma_start(
                        rb[:],
                        rdram[bh:bh + 1, :].to_broadcast((64, T)),
                    )
                    nc.vector.tensor_tensor(
                        anorm[h][:, b * T:(b + 1) * T],
                        anorm[h][:, b * T:(b + 1) * T],
                        rb[:],
                        ALU.mult,
                    )

            def a2a(b):
                a2a_in = dram.tile([NCORES * P, SL], bf16, name=f"a2a_in{b}")
                a2a_out = dram.tile([NCORES * P, SL], bf16, name=f"a2a_out{b}")
                a2a_v = a2a_in.rearrange("(j ee) t -> ee j t", j=NCORES)
                for h in range(HL):
                    nc.sync.dma_start(
                        a2a_v[64 * h:64 * h + 64],
                        anorm[h][:, b * T:(b + 1) * T]
                        .rearrange("e (j t) -> e j t", j=NCORES),
                    )
                nc.gpsimd.collective_compute(
                    "AllToAll",
                    ALU.bypass,
                    replica_groups=[list(range(NCORES))],
                    ins=[a2a_in.opt()],
                    outs=[a2a_out.opt()],
                )
                return a2a_out

            def afull_load(a2a_out):
                afull = apool.tile([P, NCORES, SL], bf16, tag="af")
                nc.gpsimd.dma_start(
                    afull[:],
                    a2a_out.rearrange("(i e) t -> e i t", i=NCORES),
                )
                return afull

            def proj_group(b, afull, tt, fc):
                ps = mm.tile([P, 512], f32, tag="mm")
                nc.tensor.matmul(
                    ps[:],
                    lhsT=ones_sb[:],
                    rhs=bp_sb[:, fc * 512:(fc + 1) * 512],
                    start=True,
                    stop=False,
                )
                for i in range(NCORES):
                    nc.tensor.matmul(
                        ps[:],
                        lhsT=afull[:, i, tt * P:(tt + 1) * P],
                        rhs=wp_sb[:, i, fc * 512:(fc + 1) * 512],
                        start=False,
                        stop=(i == NCORES - 1),
                    )
                ysb = ypool.tile([P, 512], f32, tag="ysb")
                nc.vector.tensor_copy(ysb[:], ps[:])
                nc.sync.dma_start(
                    y[b * SL + tt * P:b * SL + (tt + 1) * P,
                      fc * 512:(fc + 1) * 512],
                    ysb[:],
                )

            with tc.tile_pool(name="xslab", bufs=2) as xpool, \
                 tc.tile_pool(name="rb", bufs=1) as rbpool, \
                 tc.tile_pool(name="afull", bufs=2) as apool, \
                 tc.tile_pool(name="ysb", bufs=2) as ypool:
                for c4 in range(4):
                    qkv_chunk(0, c4)
                blocks = [(h, qc) for h in range(HL) for qc in range(NQC)]
                # b0 attention, with b1 qkv chunks interleaved into the
                # ACT-paced stretches so the PE never idles
                for i, (h, qc) in enumerate(blocks):
                    attention_block(0, h, qc)
                    if i % 2 == 1:
                        qkv_chunk(1, i // 2)
                normalize(0)
                out0 = a2a(0)
                nc.sync.dma_start(
                    wp_sb[:], wpT.rearrange("(ct p) f -> p ct f", p=P)
                )
                # b1 attention with b0 projection interleaved
                afull0 = None
                pgroups = [(tt, fc) for tt in range(SL // P)
                           for fc in range(C // 512)]
                for i, (h, qc) in enumerate(blocks):
                    attention_block(1, h, qc)
                    if i == 1:
                        afull0 = afull_load(out0)
                    if i >= 3 and i % 2 == 1:
                        tt, fc = pgroups[(i - 3) // 2]
                        proj_group(0, afull0, tt, fc)
                proj_group(0, afull0, *pgroups[3])
                normalize(1)
                out1 = a2a(1)
                pg1 = afull_load(out1)
                for tt, fc in pgroups:
                    proj_group(1, pg1, tt, fc)
    nc.compile()
    return nc


def _prep_inputs(x, W_qkv, b_qkv, W_proj, b_proj):
    x = np.asarray(x, dtype=np.float32)
    W_qkv = np.asarray(W_qkv, dtype=np.float32)
    b_qkv = np.asarray(b_qkv, dtype=np.float32)
    W_proj = np.asarray(W_proj, dtype=np.float32)
    b_proj = np.asarray(b_proj, dtype=np.float32)

    import ml_dtypes
    bf = ml_dtypes.bfloat16
    xT = np.ascontiguousarray(x.reshape(TOK, C).T).astype(bf)
    wpT = np.ascontiguousarray(W_proj.T).astype(bf)
    tri = np.triu(np.ones((P, P), dtype=np.float32)).astype(bf)
    ident = np.eye(P, dtype=np.float32).astype(bf)
    ones = np.ones((P, P), dtype=np.float32).astype(bf)

    in_maps = []
    for p in range(NCORES):
        rows = np.r_[128 * p:128 * p + 128,
                     C + 128 * p:C + 128 * p + 128,
                     2 * C + 128 * p:2 * C + 128 * p + 128]
        wslice = W_qkv[rows]                      # [384, 1024]
        bslice = np.ascontiguousarray(b_qkv[rows])
        in_maps.append({
            "xT": xT,
            "wqkvT": np.ascontiguousarray(wslice.T).astype(bf),
            "bqkv": bslice,
            "wpT": wpT,
            "bp": b_proj.astype(bf),
            "tri": tri,
            "ident": ident,
            "ones": ones,
        })
    return in_maps


def kernel(x, W_qkv, b_qkv, W_proj, b_proj, _trace=False):
    from concourse import bass_utils

    if "nc" not in _CACHE:
        _CACHE["nc"] = _build_nc()
    nc = _CACHE["nc"]
    in_maps = _prep_inputs(x, W_qkv, b_qkv, W_proj, b_proj)
    res = bass_utils.run_bass_kernel_spmd(
        nc, in_maps, core_ids=list(range(NCORES)), trace=_trace,
    )
    _CACHE["last_result"] = res
    # core p rows: [b*256 + i] = batch b, token 256*p + i
    yfull = np.empty((B, T, C), dtype=np.float32)
    for p, rmap in enumerate(res.results):
        yp = rmap["y"]
        for b in range(B):
            yfull[b, SL * p:SL * (p + 1)] = yp[b * SL:(b + 1) * SL]
    return yfull


# revision 30
# speedup vs baseline: 1.0225x; 1.0225x over previous
"""Causal self-attention (B=2, T=2048, C=1024, H=16) on 8 trn2 NeuronCores.

Sharding (Megatron-style, per spec hint):
  - tensor-parallel over heads: core p owns heads {2p, 2p+1}.  Each core
    computes Q^T/K^T/V^T for its 2 heads from the full x, then causal
    attention (streaming softmax without max-subtraction; the denominator
    comes from a ones-column appended to V).
  - per batch: an AllToAll redistributes that batch's attention outputs so
    that core p holds all 1024 channels for the batch's tokens
    [256p, 256p+256); batch 0's AllToAll and projection overlap batch 1's
    qkv/attention work.
  - projection: each core computes the full output projection for its two
    256-token slices and writes a disjoint [512, 1024] output block
    (rows b*256+i = batch b, token 256*p+i).

The emission order pipelines per batch so the (in-order) PE never waits on
the 16.8 MB x^T stream: qkv(b0) -> attention(b0) -> qkv(b1) [x tail
streams during attention(b0)] -> attention(b1) -> projections.

Layouts: x/W pre-transposed on host so contractions land on partitions;
scores computed transposed (S^T = K Q^T) so the softmax sum is a matmul
reduction and exp(S^T) feeds O^T = V^T_aug P directly, accumulated in PSUM
over k-tiles; all matmuls in float32r.
"""

import numpy as np

B, T, C, H, D = 2, 2048, 1024, 16, 64
NCORES = 8
HL = H // NCORES        # heads per core = 2
TOK = B * T             # 4096 global tokens
TSL = TOK // NCORES     # 512 output tokens per core (256 per batch)
SL = 256                # per-batch token slice per core
P = 128
CT = C // P             # 8 contraction tiles
NQC = T // 512          # 4 q-chunks per batch
NKT = T // P            # 16 k-tiles per batch
KG = 2                  # k-tiles per exp group
SCALE = D ** -0.5

_CACHE = {}


def _build_nc():
    import concourse.bass as bass
    import concourse.mybir as mybir
    from concourse import bacc
    from concourse.tile import TileContext

    f32 = mybir.dt.float32
    f32r = mybir.dt.float32r
    bf16 = mybir.dt.bfloat16
    AF = mybir.ActivationFunctionType
    ALU = mybir.AluOpType

    nc = bacc.Bacc(
        "TRN2", target_bir_lowering=False, debug=False, num_devices=NCORES
    )

    xT = nc.dram_tensor("xT", [C, TOK], bf16, kind="ExternalInput")
    wqkvT = nc.dram_tensor("wqkvT", [C, 3 * P], bf16, kind="ExternalInput")
    bqkv = nc.dram_tensor("bqkv", [3 * P], f32, kind="ExternalInput")
    wpT = nc.dram_tensor("wpT", [C, C], bf16, kind="ExternalInput")
    bp = nc.dram_tensor("bp", [C], bf16, kind="ExternalInput")
    tri = nc.dram_tensor("tri", [P, P], bf16, kind="ExternalInput")
    onesd = nc.dram_tensor("ones", [P, P], bf16, kind="ExternalInput")
    ident = nc.dram_tensor("ident", [P, P], bf16, kind="ExternalInput")
    y = nc.dram_tensor("y", [TSL, C], f32, kind="ExternalOutput")

    with TileContext(nc, num_cores=NCORES) as tc:
        from contextlib import ExitStack

        with ExitStack() as ctx:
            const = ctx.enter_context(tc.tile_pool(name="const", bufs=1))
            persist = ctx.enter_context(tc.tile_pool(name="persist", bufs=1))
            dram = ctx.enter_context(tc.tile_pool(name="dram", bufs=1, space="DRAM"))

            # ---- constants; small ones first so nothing queues behind bulk
            tri_sb = const.tile([P, P], bf16)
            id_sb = const.tile([P, P], bf16)
            bq_sb = const.tile([P, 3], f32)
            bp_sb = const.tile([1, C], bf16)
            ones_sb = const.tile([1, P], bf16)
            ones2_sb = const.tile([P, 2], bf16)
            w_sb = const.tile([P, CT, 3 * P], bf16)     # wqkvT tiles
            wp_sb = const.tile([P, CT, C], bf16)        # W_proj^T (loaded late)
            nc.gpsimd.dma_start(tri_sb[:], tri[:])
            nc.gpsimd.dma_start(id_sb[:], ident[:])
            nc.gpsimd.dma_start(bq_sb[:], bqkv.rearrange("(et p) -> p et", p=P))
            nc.gpsimd.dma_start(bp_sb[:], bp.rearrange("(o c) -> o c", o=1))
            nc.gpsimd.dma_start(ones_sb[:], onesd[0:1, :])
            nc.gpsimd.dma_start(ones2_sb[:], onesd[:, 0:2])
            nc.sync.dma_start(w_sb[:], wqkvT.rearrange("(ct p) e -> p ct e", p=P))

            # ---- persistent activations (per batch for fine-grained deps)
            qTb = [persist.tile([P, T], bf16, name=f"qT{b}") for b in range(B)]
            kTb = [persist.tile([P, T], bf16, name=f"kT{b}") for b in range(B)]
            vTb = [persist.tile([P, T], bf16, name=f"vT{b}") for b in range(B)]
            # V with ones column, per batch: [128 tok, k-tile, 2*65]
            vaugb = [persist.tile([P, NKT, 2 * 65], bf16, name=f"vaug{b}")
                     for b in range(B)]
            # A^T per local head (each head stays at partitions 0-63)
            anorm = [persist.tile([64, TOK], bf16, name=f"anorm{h}")
                     for h in range(HL)]
            ddram = dram.tile([B * HL * NQC, 512], f32)  # raw denominators
            rdram = dram.tile([B * HL, T], f32)          # reciprocals (bounce)

            pools = [
                tc.tile_pool(name="sps", bufs=2, space="PSUM"),
                tc.tile_pool(name="ops", bufs=2, space="PSUM"),
                tc.tile_pool(name="mm", bufs=2, space="PSUM"),
                tc.tile_pool(name="pT", bufs=2),
                tc.tile_pool(name="ds", bufs=2),
                tc.tile_pool(name="rp", bufs=2),
            ]
            sps, ops, mm, ppool, dspool, rppool = (
                ctx.enter_context(p) for p in pools)

            def qkv_chunk(b, tc4):
                """qkv^T for one 512-token chunk of batch b + V transposes."""
                if True:
                    xsl = xpool.tile([P, CT, 512], bf16, tag="x")
                    t0 = b * T + tc4 * 512
                    nc.sync.dma_start(
                        xsl[:],
                        xT[:, t0:t0 + 512].rearrange("(ct p) t -> p ct t", p=P),
                    )
                    for et, dstl in enumerate((qTb, kTb, vTb)):
                        ps = mm.tile([P, 512], f32, tag="mm")
                        for ct in range(CT):
                            nc.tensor.matmul(
                                ps[:],
                                lhsT=w_sb[:, ct, et * P:(et + 1) * P],
                                rhs=xsl[:, ct, :],
                                start=(ct == 0),
                                stop=(ct == CT - 1),
                            )
                        nc.vector.tensor_scalar_add(
                            dstl[b][:, tc4 * 512:(tc4 + 1) * 512],
                            ps[:],
                            bq_sb[:, et:et + 1],
                        )
                    # V^T -> V for this chunk's 4 k-tiles (PE transpose)
                    for kt in range(tc4 * 4, tc4 * 4 + 4):
                        tp = mm.tile([P, P], bf16, tag="mm")
                        nc.tensor.transpose(
                            tp[:],
                            vTb[b][:, kt * P:(kt + 1) * P],
                            id_sb[:],
                        )
                        nc.vector.tensor_copy(
                            vaugb[b][:, kt, 0:2 * 65]
                            .rearrange("p (h e) -> p h e", h=2)[:, :, 0:64],
                            tp.rearrange("p (h e) -> p h e", h=2),
                        )
                        nc.vector.tensor_copy(
                            vaugb[b][:, kt, 64:2 * 65:65], ones2_sb[:]
                        )

            def attention_qc(b, qc):
                q0 = qc * 512
                nk = 4 * qc + 4                   # causal k-tiles
                o_t = [ops.tile([65, 512], f32, tag="o", name=f"ot{hh}")
                       for hh in range(HL)]
                for g0 in range(0, nk, KG):
                    gn = min(KG, nk - g0)
                    sp = [sps.tile([P, KG * 512], f32, tag="s", name=f"sp{hh}")
                          for hh in range(HL)]
                    for j in range(gn):
                        ki = g0 + j
                        # adjacent h0/h1 matmuls use disjoint PE row groups
                        # (partitions 0-63 vs 64-127) -> run concurrently
                        for h in range(HL):
                            hp = slice(64 * h, 64 * h + 64)
                            nc.tensor.matmul(
                                sp[h][:, j * 512:(j + 1) * 512],
                                lhsT=kTb[b][hp, ki * P:(ki + 1) * P],
                                rhs=qTb[b][hp, q0:q0 + 512],
                                start=True,
                                stop=True,
                            )
                    pt = [ppool.tile([P, KG * 512], bf16, tag="p", name=f"pt{hh}")
                          for hh in range(HL)]
                    for h in range(HL):
                        nc.scalar.activation(
                            pt[h][:, 0:gn * 512],
                            sp[h][:, 0:gn * 512],
                            AF.Exp,
                            scale=SCALE,
                        )
                    for j in range(gn):
                        ki = g0 + j
                        off = ki * P - q0
                        lo = max(0, off)
                        for h in range(HL):
                            if 0 <= off:
                                nc.vector.tensor_tensor(
                                    pt[h][:, j * 512 + off:j * 512 + off + P],
                                    pt[h][:, j * 512 + off:j * 512 + off + P],
                                    tri_sb[:],
                                    ALU.mult,
                                )
                            nc.tensor.matmul(
                                o_t[h][:, lo:512],
                                lhsT=vaugb[b][:, ki, h * 65:h * 65 + 65],
                                rhs=pt[h][:, j * 512 + lo:(j + 1) * 512],
                                start=(ki == 0),
                                stop=(ki == nk - 1),
                            )
                for h in range(HL):
                    bh = b * HL + h
                    nc.vector.tensor_copy(
                        anorm[h][:, b * T + q0:b * T + q0 + 512],
                        o_t[h][0:64, :],
                    )
                    dst = dspool.tile([65, 512], f32, tag="ds")
                    nc.vector.tensor_copy(dst[64:65, :], o_t[h][64:65, :])
                    nc.sync.dma_start(
                        ddram[bh * NQC + qc:bh * NQC + qc + 1, :],
                        dst[64:65, :],
                    )

            def normalize_part(b, part):
                # tokens [1024*part, 1024*part+1024) of batch b (= 2 q-chunks)
                for h in range(HL):
                    bh = b * HL + h
                    r0 = bh * NQC + 2 * part
                    dpk = rppool.tile([32, 64], f32, tag="dpk")
                    rpk = rppool.tile([32, 64], f32, tag="rpk")
                    rsc = rppool.tile([32, 64], f32, tag="rsc")
                    nc.sync.dma_start(
                        dpk[0:16, :],
                        ddram[r0:r0 + 2, :]
                        .rearrange("u (rr f) -> (u rr) f", f=64),
                    )
                    nc.vector.reciprocal_approx_accurate(
                        rpk[0:16, :], dpk[0:16, :], rsc[0:16, :]
                    )
                    nc.sync.dma_start(
                        rdram[bh:bh + 1, 1024 * part:1024 * (part + 1)]
                        .rearrange("o (rr f) -> (o rr) f", f=64),
                        rpk[0:16, :],
                    )
                    rb = rbpool.tile([64, 1024], f32, tag="rb")
                    nc.sync.dma_start(
                        rb[:],
                        rdram[bh:bh + 1, 1024 * part:1024 * (part + 1)]
                        .to_broadcast((64, 1024)),
                    )
                    c0 = b * T + 1024 * part
                    nc.vector.tensor_tensor(
                        anorm[h][:, c0:c0 + 1024],
                        anorm[h][:, c0:c0 + 1024],
                        rb[:],
                        ALU.mult,
                    )

            def a2a_part(b, part):
                a2a_in = dram.tile([NCORES * P, P], bf16,
                                   name=f"a2a_in{b}_{part}")
                a2a_out = dram.tile([NCORES * P, P], bf16,
                                    name=f"a2a_out{b}_{part}")
                a2a_v = a2a_in.rearrange("(j ee) t -> ee j t", j=NCORES)
                for h in range(HL):
                    c0 = b * T + 1024 * part
                    nc.sync.dma_start(
                        a2a_v[64 * h:64 * h + 64],
                        anorm[h][:, c0:c0 + 1024]
                        .rearrange("e (j t) -> e j t", j=NCORES),
                    )
                nc.gpsimd.collective_compute(
                    "AllToAll",
                    ALU.bypass,
                    replica_groups=[list(range(NCORES))],
                    ins=[a2a_in.opt()],
                    outs=[a2a_out.opt()],
                )
                return a2a_out

            def afull_load(a2a_out):
                afull = apool.tile([P, NCORES, P], bf16, tag="af")
                nc.gpsimd.dma_start(
                    afull[:],
                    a2a_out.rearrange("(i e) t -> e i t", i=NCORES),
                )
                return afull

            def proj_part(b, part, afull):
                # one 128-token tile, both f-chunks
                for fc in range(C // 512):
                    ps = mm.tile([P, 512], f32, tag="mm")
                    nc.tensor.matmul(
                        ps[:],
                        lhsT=ones_sb[:],
                        rhs=bp_sb[:, fc * 512:(fc + 1) * 512],
                        start=True,
                        stop=False,
                    )
                    for i in range(NCORES):
                        nc.tensor.matmul(
                            ps[:],
                            lhsT=afull[:, i, :],
                            rhs=wp_sb[:, i, fc * 512:(fc + 1) * 512],
                            start=False,
                            stop=(i == NCORES - 1),
                        )
                    ysb = ypool.tile([P, 512], f32, tag="ysb")
                    nc.vector.tensor_copy(ysb[:], ps[:])
                    r0 = b * SL + part * P
                    nc.sync.dma_start(
                        y[r0:r0 + P, fc * 512:(fc + 1) * 512],
                        ysb[:],
                    )

            with tc.tile_pool(name="xslab", bufs=2) as xpool, \
                 tc.tile_pool(name="rb", bufs=1) as rbpool, \
                 tc.tile_pool(name="afull", bufs=2) as apool, \
                 tc.tile_pool(name="ysb", bufs=2) as ypool:
                for c4 in range(4):
                    qkv_chunk(0, c4)
                attention_qc(0, 0)
                attention_qc(0, 1)
                normalize_part(0, 0)
                out00 = a2a_part(0, 0)
                qkv_chunk(1, 0)
                qkv_chunk(1, 1)
                attention_qc(0, 2)
                attention_qc(0, 3)
                normalize_part(0, 1)
                out01 = a2a_part(0, 1)
                qkv_chunk(1, 2)
                qkv_chunk(1, 3)
                nc.sync.dma_start(
                    wp_sb[:], wpT.rearrange("(ct p) f -> p ct f", p=P)
                )
                attention_qc(1, 0)
                af00 = afull_load(out00)
                proj_part(0, 0, af00)
                attention_qc(1, 1)
                af01 = afull_load(out01)
                proj_part(0, 1, af01)
                normalize_part(1, 0)
                out10 = a2a_part(1, 0)
                attention_qc(1, 2)
                af10 = afull_load(out10)
                proj_part(1, 0, af10)
                attention_qc(1, 3)
                normalize_part(1, 1)
                out11 = a2a_part(1, 1)
                af11 = afull_load(out11)
                proj_part(1, 1, af11)
    nc.compile()
    return nc


def _prep_inputs(x, W_qkv, b_qkv, W_proj, b_proj):
    x = np.asarray(x, dtype=np.float32)
    W_qkv = np.asarray(W_qkv, dtype=np.float32)
    b_qkv = np.asarray(b_qkv, dtype=np.float32)
    W_proj = np.asarray(W_proj, dtype=np.float32)
    b_proj = np.asarray(b_proj, dtype=np.float32)

    import ml_dtypes
    bf = ml_dtypes.bfloat16
    xT = np.ascontiguousarray(x.reshape(TOK, C).T).astype(bf)
    wpT = np.ascontiguousarray(W_proj.T).astype(bf)
    tri = np.triu(np.ones((P, P), dtype=np.float32)).astype(bf)
    ident = np.eye(P, dtype=np.float32).astype(bf)
    ones = np.ones((P, P), dtype=np.float32).astype(bf)

    in_maps = []
    for p in range(NCORES):
        rows = np.r_[128 * p:128 * p + 128,
                     C + 128 * p:C + 128 * p + 128,
                     2 * C + 128 * p:2 * C + 128 * p + 128]
        wslice = W_qkv[rows]                      # [384, 1024]
        bslice = np.ascontiguousarray(b_qkv[rows])
        in_maps.append({
            "xT": xT,
            "wqkvT": np.ascontiguousarray(wslice.T).astype(bf),
            "bqkv": bslice,
            "wpT": wpT,
            "bp": b_proj.astype(bf),
            "tri": tri,
            "ident": ident,
            "ones": ones,
        })
    return in_maps


def kernel(x, W_qkv, b_qkv, W_proj, b_proj, _trace=False):
    from concourse import bass_utils

    if "nc" not in _CACHE:
        _CACHE["nc"] = _build_nc()
    nc = _CACHE["nc"]
    in_maps = _prep_inputs(x, W_qkv, b_qkv, W_proj, b_proj)
    res = bass_utils.run_bass_kernel_spmd(
        nc, in_maps, core_ids=list(range(NCORES)), trace=_trace,
    )
    _CACHE["last_result"] = res
    # core p rows: [b*256 + part*128 + i] = batch b, token
    # b*2048 + part*1024 + 128*p + i
    yfull = np.empty((B, T, C), dtype=np.float32)
    for p, rmap in enumerate(res.results):
        yp = rmap["y"]
        for b in range(B):
            for part in range(2):
                g0 = part * 1024 + 128 * p
                r0 = b * SL + part * P
                yfull[b, g0:g0 + P] = yp[r0:r0 + P]
    return yfull
